# revision 21
# baseline (speedup 1.0000x reference)
"""Distributed Trainium2 kernel for the per-agent trajectory attention module.

Math (per reference):
    q = received_messages @ Wq + bq                    [512, 512]
    k = taus @ Wk + bk ; v = taus @ Wv + bv            [16*512, 512/64]
    scores[i, t] = dot(q[i], k[t, i]) / sqrt(512)
    messages[i] = sum_t softmax(scores)[i, t] * v[t, i]  [512, 64]

Sharding over 8 NeuronCores:
  - q: tensor-parallel over the 32768 msg dim (4096 per core), partial q for
    ALL 512 agents per core, one f32 ReduceScatter(add) over the agent axis ->
    each core holds q for its own 64 agents.  The 1/sqrt(H) scale is folded
    into Wq on the host and bq*scale/8 into each core's pre-collective drain,
    so nothing downstream of the collective needs a fixup op.  bk drops out
    of softmax exactly (per-agent constant shift); bv is added at the end.
  - k/v/attention: data-parallel over agents (64 per core).
  - matmul operands stored/computed in bf16 (halves the DMA roofline);
    PSUM accumulation and everything after the matmuls is f32.

Scheduling notes (v2):
  - every input tile is resident in SBUF; the bulk loads all go on the sync
    HWDGE ring in priority order (rm0, wq, rm1-3, traj, wk, wv) so the q
    path -- which gates the ReduceScatter -- gets the HBM bandwidth first,
    and the latency-critical small transfers (rs_in drains, q2, merge, out)
    ride the otherwise-idle scalar HWDGE ring.
  - softmax skips the max-subtraction (scores are bounded |s| < ~20 for this
    problem so exp stays comfortably inside f32), which removes the
    cross-partition score gather; the odd/even trajectory-step halves are
    instead merged once at the very end: the unnormalized exp-weighted v sum
    and the exp row-sum live in one [128, 1+DV] tile, one tiny SBUF->SBUF
    DMA folds the upper partition half down, and a single
    (sum*recip)+bv fixup produces the output.
"""

import math

import numpy as np

T = 16
N_AGENTS = 512
TAU = 2048
MSG = 32768
HID = 512
DV = 64

NC = 8
AG = N_AGENTS // NC  # 64 agents per core
GS = 8  # group size: the q reduce-scatter spans all 8 cores
NG = NC // GS
GAG = N_AGENTS // NG  # agents per group (512)
MS = MSG // GS  # 4096 msg columns per core
KQ = MS // 128  # 32 contraction chunks for q
KT = TAU // 128  # 16 contraction chunks for k/v
RT = (T * AG) // 128  # 8 row-tiles of taus per core (128 rows each)

SCALE = 1.0 / math.sqrt(HID)

# compute/storage dtype for the big matmul operands: "f32r" (full f32 storage,
# tf32-ish matmul precision) or "bf16" (half the DMA bytes, bf16 matmuls)
DTYPE = "bf16"
WARMUP_MMS = 48  # dummy matmuls to lift the PE HAM throttle before real work
RS_DT = "f16"  # collective wire dtype: "f32" or "f16" (CCE bf16 is broken on HW)

_CACHE = {}

# set by test harness: run with trace and stash exec time here
TRACE = False
TRACE_CORES = None
STITCH = False
LAST_EXEC_NS = None
LAST_RESULTS = None


def _build():
    import concourse.bacc as bacc
    import concourse.mybir as mybir
    import concourse.tile as tile
    from concourse.tile import add_dep_helper

    f32 = mybir.dt.float32
    f32r = mybir.dt.float32r if DTYPE == "f32r" else mybir.dt.bfloat16
    add = mybir.AluOpType.add
    mult = mybir.AluOpType.mult

    nc = bacc.Bacc("TRN2", target_bir_lowering=False, debug=False, num_devices=NC)

    # inputs (per-core shards, pre-packed host-side; layout [128, kc, n])
    rm_d = nc.dram_tensor("rm", [GAG // 128, 128, KQ, 128], f32r, kind="ExternalInput")
    wq_d = nc.dram_tensor("wq", [128, KQ, HID], f32r, kind="ExternalInput")
    traj_d = nc.dram_tensor("traj", [RT, 128, KT, 128], f32r, kind="ExternalInput")
    wk_d = nc.dram_tensor("wk", [128, KT, HID], f32r, kind="ExternalInput")
    wv_d = nc.dram_tensor("wv", [128, KT, DV], f32r, kind="ExternalInput")
    bqs_d = nc.dram_tensor("bqs", [128, HID], f32, kind="ExternalInput")  # bq * SCALE
    bv_d = nc.dram_tensor("bv", [AG, DV], f32, kind="ExternalInput")
    out_d = nc.dram_tensor("out", [AG, DV], f32, kind="ExternalOutput")

    # collective wire dtype (CCE bf16 reduce is broken on HW; fp16 has the
    # mantissa for this data -- q values are O(10) -- and halves the payload)
    cdt = mybir.dt.float16 if RS_DT == "f16" else f32
    # the reduce-scatter is split unevenly: A carries just the first m-tile
    # (triggered as early as possible -- it absorbs the one-time collective
    # entry latency), B carries the remaining three and pipelines behind A
    # with no extra entry fee.  The host-side agent permutation makes each
    # half-RS scatter rows so core c receives its own agents (A: local
    # 0..15, B: local 16..63)
    rs_inA = nc.dram_tensor("rs_inA", [128, HID], cdt)
    rs_outA = nc.dram_tensor("rs_outA", [16, HID], cdt)
    rs_inB = nc.dram_tensor("rs_inB", [384, HID], cdt)
    rs_outB = nc.dram_tensor("rs_outB", [48, HID], cdt)


    with tile.TileContext(nc) as tc:
        with (
            tc.tile_pool(name="res", bufs=1) as res,
            tc.tile_pool(name="work", bufs=2) as work,
            tc.tile_pool(name="qps", bufs=6, space="PSUM") as qps,
            tc.tile_pool(name="vps", bufs=2, space="PSUM") as vps,
        ):
            # ---------------- PE warm-up (HAM unthrottle) ----------------
            if WARMUP_MMS:
                wz = res.tile([128, 128], f32r)
                nc.gpsimd.memset(wz[:], 0.0)
                wacc = qps.tile([128, 512], f32, tag="acc", name="warm_acc")
                for i in range(WARMUP_MMS):
                    nc.tensor.matmul(
                        wacc[:, 0:128],
                        wz[:],
                        wz[:],
                        start=(i == 0),
                        stop=(i == WARMUP_MMS - 1),
                    )

            # ---------------- resident tensors ----------------
            # small latency-insensitive loads ride the scalar ring
            bqs_sb = res.tile([128, HID], f32)
            bv_sb = res.tile([AG, DV], f32)
            nc.scalar.dma_start(bqs_sb[:], bqs_d[:])
            nc.scalar.dma_start(bv_sb[:], bv_d[:])

            # bulk loads on the sync ring, q-path first
            wq_sb = res.tile([128, KQ, HID], f32r)
            rm_tiles = [
                res.tile([128, KQ, 128], f32r, name=f"rm_sb{m}")
                for m in range(GAG // 128)
            ]
            tj_tiles = [
                res.tile([128, KT, 128], f32r, name=f"tj{rt}") for rt in range(RT)
            ]
            wk_sb = res.tile([128, KT, HID], f32r, name="wk_sb")
            wv_sb = res.tile([128, KT, DV], f32r, name="wv_sb")

            nc.sync.dma_start(rm_tiles[0][:], rm_d[0])
            for w4 in range(8):
                nc.sync.dma_start(
                    wq_sb[:, w4 * (KQ // 8) : (w4 + 1) * (KQ // 8), :],
                    wq_d[:, w4 * (KQ // 8) : (w4 + 1) * (KQ // 8), :],
                )
            for m in range(1, GAG // 128):
                nc.sync.dma_start(rm_tiles[m][:], rm_d[m])
            # wk/wv before the traj tiles: the first k matmul needs them
            nc.sync.dma_start(wk_sb[:], wk_d[:])
            nc.sync.dma_start(wv_sb[:], wv_d[:])
            for rt in range(RT):
                nc.sync.dma_start(tj_tiles[rt][:], traj_d[rt])

            # ------- q phase: partial q for the group's agents -------
            def rs_launch(in_t, out_t):
                nc.gpsimd.collective_compute(
                    "ReduceScatter",
                    add,
                    replica_groups=[list(range(NC))],
                    ins=[in_t.ap().opt()],
                    outs=[out_t.ap().opt()],
                )

            for m in range(GAG // 128):
                rm_sb = rm_tiles[m]
                qacc = qps.tile([128, HID], f32, tag="acc")
                for kc in range(KQ):
                    nc.tensor.matmul(
                        qacc[:],
                        rm_sb[:, kc, :],
                        wq_sb[:, kc, :],
                        start=(kc == 0),
                        stop=(kc == KQ - 1),
                    )
                qdr = work.tile([128, HID], cdt, tag="qdr")
                # qacc is already scaled (Wq pre-scaled on host); add bq*SCALE/NC
                # here so the ReduceScatter sum carries the bias exactly once
                nc.vector.scalar_tensor_tensor(
                    qdr[:], qacc[:], 1.0, bqs_sb[:], mult, add
                )
                if m == 0:
                    nc.scalar.dma_start(rs_inA[:], qdr[:])
                    rs_launch(rs_inA, rs_outA)
                else:
                    nc.scalar.dma_start(
                        rs_inB[(m - 1) * 128 : m * 128, :], qdr[:]
                    )
                    if m == 3:
                        rs_launch(rs_inB, rs_outB)

            # local q, duplicated into both partition halves, kept in the fp16
            # wire format.  A-half loads fly while the B collective runs.
            q2 = res.tile([128, HID], cdt)
            nc.scalar.dma_start(q2[0:16, :], rs_outA[:])
            nc.scalar.dma_start(q2[AG : AG + 16, :], rs_outA[:])
            nc.scalar.dma_start(q2[16:AG, :], rs_outB[:])
            nc.scalar.dma_start(q2[AG + 16 : 128, :], rs_outB[:])

            # ---------------- k/v phase + scores ----------------
            v_sb = res.tile([128, RT, DV], f32)
            k_sb = res.tile([128, RT, HID], cdt)
            s_scr = res.tile([128, RT], f32)

            def kv_tile(rt):
                tj_sb = tj_tiles[rt]
                kacc = qps.tile([128, HID], f32, tag="acc", name=f"kacc{rt}")
                for kc in range(KT):
                    nc.tensor.matmul(
                        kacc[:],
                        tj_sb[:, kc, :],
                        wk_sb[:, kc, :],
                        start=(kc == 0),
                        stop=(kc == KT - 1),
                    )
                vacc = vps.tile([128, DV], f32, tag="vacc", name=f"vacc{rt}")
                for kc in range(KT):
                    nc.tensor.matmul(
                        vacc[:],
                        tj_sb[:, kc, :],
                        wv_sb[:, kc, :],
                        start=(kc == 0),
                        stop=(kc == KT - 1),
                    )
                # park k in SBUF so the PSUM bank frees without waiting on q2
                nc.vector.tensor_copy(k_sb[:, rt, :], kacc[:])
                return nc.vector.tensor_copy(v_sb[:, rt, :], vacc[:])

            def score_tile(rt):
                prod = work.tile([128, HID], cdt, tag="ttr", name=f"prod{rt}")
                return nc.vector.scalar_tensor_tensor(
                    prod[:],
                    k_sb[:, rt, :],
                    1.0,
                    q2[:],
                    mult,
                    mult,
                    accum_out=s_scr[:, rt : rt + 1],
                )

            # PE only needs drains rt0..5 to keep its PSUM slots rotating, so
            # emit scores 0..5 right after drain 5 — they fire the moment the
            # collective lands instead of waiting for the last two drains.
            last_drain5 = None
            for rt in range(6):
                last_drain5 = kv_tile(rt)
            first_score = score_tile(0)
            add_dep_helper(
                first_score.ins,
                last_drain5.ins,
                sync=False,
                reason="drains 0-5 before any RS-gated score",
            )
            for rt in range(1, 6):
                score_tile(rt)
            for rt in range(6, RT):
                kv_tile(rt)
            for rt in range(6, RT):
                score_tile(rt)

            # ------- softmax over t (16 steps per agent), no max-shift -------
            # scores for this problem are bounded (|s| < ~20), so exp is safe
            # in f32 without the running-max subtraction; the normalization
            # happens once at the very end.
            ex = res.tile([128, RT], f32)
            comb = res.tile([128, 1 + DV], f32)  # col 0: sum(exp); 1..: sum(exp*v)
            nc.scalar.activation(
                ex[:],
                s_scr[:],
                mybir.ActivationFunctionType.Exp,
                accum_out=comb[:, 0:1],
            )

            # unnormalized exp-weighted sum of v
            nc.vector.tensor_scalar_mul(comb[:, 1:], v_sb[:, 0, :], ex[:, 0:1])
            for rt in range(1, RT):
                nc.vector.scalar_tensor_tensor(
                    comb[:, 1:], v_sb[:, rt, :], ex[:, rt : rt + 1], comb[:, 1:],
                    mult, add,
                )

            # fold the odd-step partition half onto the even half, normalize,
            # add bv, and write out — one tiny DMA, three DVE ops, one store
            combU = res.tile([AG, 1 + DV], f32)
            nc.scalar.dma_start(combU[:], comb[AG:128, :])
            tot = res.tile([AG, 1 + DV], f32)
            nc.vector.tensor_tensor(tot[:], comb[0:AG, :], combU[:], add)
            rcp = res.tile([AG, 1], f32)
            nc.vector.reciprocal(rcp[:], tot[:, 0:1])
            mfin = res.tile([AG, DV], f32)
            nc.vector.scalar_tensor_tensor(
                mfin[:], tot[:, 1:], rcp[:, 0:1], bv_sb[:], mult, add
            )
            nc.scalar.dma_start(out_d[:], mfin[:])

    nc.compile()
    return nc


# packed q row r -> original agent, chosen so the uneven split
# reduce-scatters land core c's own agents:
# A ([128, HID] -> 16 rows/core): rows c*16+j -> agent 64c+j
# B ([384, HID] -> 48 rows/core): rows c*48+j -> agent 64c+16+j
_AGENT_PERM = np.array(
    [
        (64 * (r // 16) + r % 16)
        if r < 128
        else (64 * ((r - 128) // 48) + 16 + (r - 128) % 48)
        for r in range(N_AGENTS)
    ],
    dtype=np.int64,
)


def _cdt():
    if DTYPE == "bf16":
        import ml_dtypes

        return ml_dtypes.bfloat16
    return np.float32


def _pack(a, kchunks, inner):
    # [K, N] -> [128, K//128, N] with the contraction dim on partitions
    return np.ascontiguousarray(
        a.reshape(kchunks, 128, inner).transpose(1, 0, 2), dtype=_cdt()
    )


def _make_in_maps(
    imagined_trajectory, received_messages, Wq, bq, Wk, bk, Wv, bv
):
    imagined_trajectory = np.asarray(imagined_trajectory, dtype=np.float32)
    received_messages = np.asarray(received_messages, dtype=np.float32)
    Wq = np.asarray(Wq, dtype=np.float32)
    bq = np.asarray(bq, dtype=np.float32)
    Wk = np.asarray(Wk, dtype=np.float32)
    Wv = np.asarray(Wv, dtype=np.float32)
    bv = np.asarray(bv, dtype=np.float32)

    wk_p = _pack(Wk, KT, HID)
    wv_p = _pack(Wv, KT, DV)
    bqs = np.ascontiguousarray(
        np.broadcast_to(bq * SCALE / NC, (128, HID)), dtype=np.float32
    )
    bv_r = np.ascontiguousarray(np.broadcast_to(bv, (AG, DV)), dtype=np.float32)

    in_maps = []
    for c in range(NC):
        g, s = c // GS, c % GS
        gslice = slice(g * GAG, (g + 1) * GAG)
        mslice = slice(s * MS, (s + 1) * MS)
        rm_t = received_messages[gslice, mslice].T[:, _AGENT_PERM]  # [4096, 512]
        wq_sh = Wq[mslice, :] * SCALE  # [4096, 512], pre-scaled
        taus = imagined_trajectory[:, c * AG : (c + 1) * AG, :].reshape(T * AG, TAU)
        traj_t = taus.T  # [2048, 1024]
        rm_p = np.ascontiguousarray(
            rm_t.reshape(KQ, 128, GAG // 128, 128).transpose(2, 1, 0, 3),
            dtype=_cdt(),
        )
        traj_p = np.ascontiguousarray(
            traj_t.reshape(KT, 128, RT, 128).transpose(2, 1, 0, 3), dtype=_cdt()
        )
        in_maps.append(
            {
                "rm": rm_p,
                "wq": _pack(wq_sh, KQ, HID),
                "traj": traj_p,
                "wk": wk_p,
                "wv": wv_p,
                "bqs": bqs,
                "bv": bv_r,
            }
        )
    return in_maps


def kernel(
    imagined_trajectory,
    received_messages,
    Wq,
    bq,
    Wk,
    bk,
    Wv,
    bv,
):
    global LAST_EXEC_NS, LAST_RESULTS
    from concourse.bass_utils import run_bass_kernel_spmd

    if "nc" not in _CACHE:
        _CACHE["nc"] = _build()
    nc = _CACHE["nc"]

    in_maps = _make_in_maps(
        imagined_trajectory, received_messages, Wq, bq, Wk, bk, Wv, bv
    )

    res = run_bass_kernel_spmd(
        nc,
        in_maps,
        core_ids=list(range(NC)),
        trace=TRACE,
        trace_cores=TRACE_CORES,
        stitch_traces=STITCH,
    )
    LAST_EXEC_NS = res.exec_time_ns
    LAST_RESULTS = res
    out = np.concatenate([res.results[c]["out"] for c in range(NC)], axis=0)
    return out.astype(np.float32)


# revision 25
# speedup vs baseline: 1.0813x; 1.0813x over previous
"""Distributed Trainium2 kernel for the per-agent trajectory attention module.

Math (per reference):
    q = received_messages @ Wq + bq                    [512, 512]
    k = taus @ Wk + bk ; v = taus @ Wv + bv            [16*512, 512/64]
    scores[i, t] = dot(q[i], k[t, i]) / sqrt(512)
    messages[i] = sum_t softmax(scores)[i, t] * v[t, i]  [512, 64]

Sharding over 8 NeuronCores:
  - q: tensor-parallel over the 32768 msg dim (4096 per core), partial q for
    ALL 512 agents per core, one f32 ReduceScatter(add) over the agent axis ->
    each core holds q for its own 64 agents.  The 1/sqrt(H) scale is folded
    into Wq on the host and bq*scale/8 into each core's pre-collective drain,
    so nothing downstream of the collective needs a fixup op.  bk drops out
    of softmax exactly (per-agent constant shift); bv is added at the end.
  - k/v/attention: data-parallel over agents (64 per core).
  - matmul operands stored/computed in bf16 (halves the DMA roofline);
    PSUM accumulation and everything after the matmuls is f32.

Scheduling notes (v2):
  - every input tile is resident in SBUF; the bulk loads all go on the sync
    HWDGE ring in priority order (rm0, wq, rm1-3, traj, wk, wv) so the q
    path -- which gates the ReduceScatter -- gets the HBM bandwidth first,
    and the latency-critical small transfers (rs_in drains, q2, merge, out)
    ride the otherwise-idle scalar HWDGE ring.
  - softmax skips the max-subtraction (scores are bounded |s| < ~20 for this
    problem so exp stays comfortably inside f32), which removes the
    cross-partition score gather; the odd/even trajectory-step halves are
    instead merged once at the very end: the unnormalized exp-weighted v sum
    and the exp row-sum live in one [128, 1+DV] tile, one tiny SBUF->SBUF
    DMA folds the upper partition half down, and a single
    (sum*recip)+bv fixup produces the output.
"""

import math

import numpy as np

T = 16
N_AGENTS = 512
TAU = 2048
MSG = 32768
HID = 512
DV = 64

NC = 8
AG = N_AGENTS // NC  # 64 agents per core
GS = 8  # group size: the q reduce-scatter spans all 8 cores
NG = NC // GS
GAG = N_AGENTS // NG  # agents per group (512)
MS = MSG // GS  # 4096 msg columns per core
KQ = MS // 128  # 32 contraction chunks for q
KT = TAU // 128  # 16 contraction chunks for k/v
RT = (T * AG) // 128  # 8 row-tiles of taus per core (128 rows each)

SCALE = 1.0 / math.sqrt(HID)

# compute/storage dtype for the big matmul operands: "f32r" (full f32 storage,
# tf32-ish matmul precision) or "bf16" (half the DMA bytes, bf16 matmuls)
DTYPE = "bf16"
WARMUP_MMS = 48  # dummy matmuls to lift the PE HAM throttle before real work
RS_DT = "f16"  # collective wire dtype: "f32" or "f16" (CCE bf16 is broken on HW)

_CACHE = {}

# set by test harness: run with trace and stash exec time here
TRACE = False
TRACE_CORES = None
STITCH = False
LAST_EXEC_NS = None
LAST_RESULTS = None


def _build():
    import concourse.bacc as bacc
    import concourse.mybir as mybir
    import concourse.tile as tile
    from concourse.tile import add_dep_helper

    f32 = mybir.dt.float32
    f32r = mybir.dt.float32r if DTYPE == "f32r" else mybir.dt.bfloat16
    add = mybir.AluOpType.add
    mult = mybir.AluOpType.mult

    nc = bacc.Bacc("TRN2", target_bir_lowering=False, debug=False, num_devices=NC)

    # inputs (per-core shards, pre-packed host-side; layout [128, kc, n])
    rm_d = nc.dram_tensor("rm", [GAG // 128, 128, KQ, 128], f32r, kind="ExternalInput")
    wq_d = nc.dram_tensor("wq", [128, KQ, HID], f32r, kind="ExternalInput")
    traj_d = nc.dram_tensor("traj", [RT, 128, KT, 128], f32r, kind="ExternalInput")
    wk_d = nc.dram_tensor("wk", [128, KT, HID], f32r, kind="ExternalInput")
    wv_d = nc.dram_tensor("wv", [128, KT, DV], f32r, kind="ExternalInput")
    bqs_d = nc.dram_tensor("bqs", [128, HID], f32, kind="ExternalInput")  # bq * SCALE
    bv_d = nc.dram_tensor("bv", [AG, DV], f32, kind="ExternalInput")
    out_d = nc.dram_tensor("out", [AG, DV], f32, kind="ExternalOutput")

    # collective wire dtype (CCE bf16 reduce is broken on HW; fp16 has the
    # mantissa for this data -- q values are O(10) -- and halves the payload)
    cdt = mybir.dt.float16 if RS_DT == "f16" else f32
    # the reduce-scatter is split in two so the first half can fly while the
    # second half of the q matmuls still runs; the host-side agent
    # permutation makes each half-RS scatter rows so core c receives its own
    # agents (A: local 0..31, B: local 32..63)
    rs_inA = nc.dram_tensor("rs_inA", [GAG // 2, HID], cdt)
    rs_outA = nc.dram_tensor("rs_outA", [AG // 2, HID], cdt)
    rs_inB = nc.dram_tensor("rs_inB", [GAG // 2, HID], cdt)
    rs_outB = nc.dram_tensor("rs_outB", [AG // 2, HID], cdt)


    with tile.TileContext(nc) as tc:
        with (
            tc.tile_pool(name="res", bufs=1) as res,
            tc.tile_pool(name="work", bufs=2) as work,
            tc.tile_pool(name="qps", bufs=6, space="PSUM") as qps,
            tc.tile_pool(name="vps", bufs=2, space="PSUM") as vps,
        ):
            # ---------------- PE warm-up (HAM unthrottle) ----------------
            if WARMUP_MMS:
                wz = res.tile([128, 128], f32r)
                nc.gpsimd.memset(wz[:], 0.0)
                wacc = qps.tile([128, 512], f32, tag="acc", name="warm_acc")
                for i in range(WARMUP_MMS):
                    nc.tensor.matmul(
                        wacc[:, 0:128],
                        wz[:],
                        wz[:],
                        start=(i == 0),
                        stop=(i == WARMUP_MMS - 1),
                    )

            # ---------------- resident tensors ----------------
            # small latency-insensitive loads ride the scalar ring
            bqs_sb = res.tile([128, HID], f32)
            bv_sb = res.tile([AG, DV], f32)
            nc.scalar.dma_start(bqs_sb[:], bqs_d[:])
            nc.scalar.dma_start(bv_sb[:], bv_d[:])

            # bulk loads on the sync ring, q-path first
            wq_sb = res.tile([128, KQ, HID], f32r)
            rm_tiles = [
                res.tile([128, KQ, 128], f32r, name=f"rm_sb{m}")
                for m in range(GAG // 128)
            ]
            tj_tiles = [
                res.tile([128, KT, 128], f32r, name=f"tj{rt}") for rt in range(RT)
            ]
            wk_sb = res.tile([128, KT, HID], f32r, name="wk_sb")
            wv_sb = res.tile([128, KT, DV], f32r, name="wv_sb")

            nc.sync.dma_start(rm_tiles[0][:], rm_d[0])
            for w4 in range(8):
                nc.sync.dma_start(
                    wq_sb[:, w4 * (KQ // 8) : (w4 + 1) * (KQ // 8), :],
                    wq_d[:, w4 * (KQ // 8) : (w4 + 1) * (KQ // 8), :],
                )
            for m in range(1, GAG // 128):
                nc.sync.dma_start(rm_tiles[m][:], rm_d[m])
            # wk/wv before the traj tiles: the first k matmul needs them
            nc.sync.dma_start(wk_sb[:], wk_d[:])
            nc.sync.dma_start(wv_sb[:], wv_d[:])
            for rt in range(RT):
                nc.sync.dma_start(tj_tiles[rt][:], traj_d[rt])

            # ------- q phase: partial q for the group's agents -------
            def rs_launch(in_t, out_t):
                nc.gpsimd.collective_compute(
                    "ReduceScatter",
                    add,
                    replica_groups=[list(range(NC))],
                    ins=[in_t.ap().opt()],
                    outs=[out_t.ap().opt()],
                )

            for m in range(GAG // 128):
                rm_sb = rm_tiles[m]
                qacc = qps.tile([128, HID], f32, tag="acc")
                for kc in range(KQ):
                    nc.tensor.matmul(
                        qacc[:],
                        rm_sb[:, kc, :],
                        wq_sb[:, kc, :],
                        start=(kc == 0),
                        stop=(kc == KQ - 1),
                    )
                qdr = work.tile([128, HID], cdt, tag="qdr")
                # qacc is already scaled (Wq pre-scaled on host); add bq*SCALE/NC
                # here so the ReduceScatter sum carries the bias exactly once
                nc.vector.scalar_tensor_tensor(
                    qdr[:], qacc[:], 1.0, bqs_sb[:], mult, add
                )
                half_in = rs_inA if m < 2 else rs_inB
                nc.scalar.dma_start(
                    half_in[(m % 2) * 128 : (m % 2 + 1) * 128, :], qdr[:]
                )
                if m == 1:
                    rs_launch(rs_inA, rs_outA)
                elif m == 3:
                    rs_launch(rs_inB, rs_outB)

            # local q, duplicated into both partition halves, kept in the fp16
            # wire format.  A-half loads fly while the B collective runs.
            q2 = res.tile([128, HID], cdt)
            nc.scalar.dma_start(q2[0 : AG // 2, :], rs_outA[:])
            nc.scalar.dma_start(q2[AG : AG + AG // 2, :], rs_outA[:])
            nc.scalar.dma_start(q2[AG // 2 : AG, :], rs_outB[:])
            nc.scalar.dma_start(q2[AG + AG // 2 : 128, :], rs_outB[:])

            # ---------------- k/v phase + scores ----------------
            # v_sb column 0 is a constant 1.0: the exp-weighted v chain then
            # accumulates sum(exp) in comb[:,0] for free
            v_sb = res.tile([128, RT, 1 + DV], f32)
            nc.gpsimd.memset(v_sb[:, :, 0:1], 1.0)
            k_sb = res.tile([128, RT, HID], cdt)
            s_scr = res.tile([128, RT], f32)

            def kv_tile(rt):
                tj_sb = tj_tiles[rt]
                kacc = qps.tile([128, HID], f32, tag="acc", name=f"kacc{rt}")
                for kc in range(KT):
                    nc.tensor.matmul(
                        kacc[:],
                        tj_sb[:, kc, :],
                        wk_sb[:, kc, :],
                        start=(kc == 0),
                        stop=(kc == KT - 1),
                    )
                vacc = vps.tile([128, DV], f32, tag="vacc", name=f"vacc{rt}")
                for kc in range(KT):
                    nc.tensor.matmul(
                        vacc[:],
                        tj_sb[:, kc, :],
                        wv_sb[:, kc, :],
                        start=(kc == 0),
                        stop=(kc == KT - 1),
                    )
                # park k in SBUF so the PSUM bank frees without waiting on q2
                nc.vector.tensor_copy(k_sb[:, rt, :], kacc[:])
                return nc.vector.tensor_copy(v_sb[:, rt, 1:], vacc[:])

            def score_tile(rt):
                prod = work.tile([128, HID], cdt, tag="ttr", name=f"prod{rt}")
                return nc.vector.scalar_tensor_tensor(
                    prod[:],
                    k_sb[:, rt, :],
                    1.0,
                    q2[:],
                    mult,
                    mult,
                    accum_out=s_scr[:, rt : rt + 1],
                )

            # PE only needs drains rt0..5 to keep its PSUM slots rotating, so
            # emit scores 0..5 right after drain 5 — they fire the moment the
            # collective lands instead of waiting for the last two drains.
            last_drain5 = None
            for rt in range(6):
                last_drain5 = kv_tile(rt)
            first_score = score_tile(0)
            add_dep_helper(
                first_score.ins,
                last_drain5.ins,
                sync=False,
                reason="drains 0-5 before any RS-gated score",
            )
            for rt in range(1, 6):
                score_tile(rt)
            for rt in range(6, RT):
                kv_tile(rt)
            for rt in range(6, RT):
                score_tile(rt)

            # ------- softmax over t (16 steps per agent), no max-shift -------
            # scores for this problem are bounded (|s| < ~20), so exp is safe
            # in f32 without the running-max subtraction; the normalization
            # happens once at the very end.  exp runs per-column on ScalarE
            # and the exp-weighted v chain on GpSimd, so both pipeline behind
            # the score dots on VectorE instead of queueing after them.
            ex = res.tile([128, RT], f32)
            comb = res.tile([128, 1 + DV], f32)  # col 0: sum(exp); 1..: sum(exp*v)
            for rt in range(RT):
                nc.scalar.activation(
                    ex[:, rt : rt + 1],
                    s_scr[:, rt : rt + 1],
                    mybir.ActivationFunctionType.Exp,
                )
            nc.gpsimd.tensor_scalar_mul(comb[:], v_sb[:, 0, :], ex[:, 0:1])
            for rt in range(1, RT):
                nc.gpsimd.scalar_tensor_tensor(
                    comb[:], v_sb[:, rt, :], ex[:, rt : rt + 1], comb[:],
                    mult, add,
                )

            # fold the odd-step partition half onto the even half, normalize,
            # add bv, and write out — one tiny DMA, three DVE ops, one store
            combU = res.tile([AG, 1 + DV], f32)
            nc.gpsimd.dma_start(combU[:], comb[AG:128, :])
            tot = res.tile([AG, 1 + DV], f32)
            nc.vector.tensor_tensor(tot[:], comb[0:AG, :], combU[:], add)
            rcp = res.tile([AG, 1], f32)
            nc.vector.reciprocal(rcp[:], tot[:, 0:1])
            mfin = res.tile([AG, DV], f32)
            nc.vector.scalar_tensor_tensor(
                mfin[:], tot[:, 1:], rcp[:, 0:1], bv_sb[:], mult, add
            )
            nc.scalar.dma_start(out_d[:], mfin[:])

    nc.compile()
    return nc


# packed q row r -> original agent, chosen so each half reduce-scatter
# ([256, HID] over 8 ranks -> 32 rows per core) lands core c's own agents:
# half A rows c*32+j -> agent 64c+j, half B rows c*32+j -> agent 64c+32+j
_AGENT_PERM = np.array(
    [
        64 * ((r % 256) // 32) + 32 * (r // 256) + (r % 32)
        for r in range(N_AGENTS)
    ],
    dtype=np.int64,
)


def _cdt():
    if DTYPE == "bf16":
        import ml_dtypes

        return ml_dtypes.bfloat16
    return np.float32


def _pack(a, kchunks, inner):
    # [K, N] -> [128, K//128, N] with the contraction dim on partitions
    return np.ascontiguousarray(
        a.reshape(kchunks, 128, inner).transpose(1, 0, 2), dtype=_cdt()
    )


def _make_in_maps(
    imagined_trajectory, received_messages, Wq, bq, Wk, bk, Wv, bv
):
    imagined_trajectory = np.asarray(imagined_trajectory, dtype=np.float32)
    received_messages = np.asarray(received_messages, dtype=np.float32)
    Wq = np.asarray(Wq, dtype=np.float32)
    bq = np.asarray(bq, dtype=np.float32)
    Wk = np.asarray(Wk, dtype=np.float32)
    Wv = np.asarray(Wv, dtype=np.float32)
    bv = np.asarray(bv, dtype=np.float32)

    wk_p = _pack(Wk, KT, HID)
    wv_p = _pack(Wv, KT, DV)
    bqs = np.ascontiguousarray(
        np.broadcast_to(bq * SCALE / NC, (128, HID)), dtype=np.float32
    )
    bv_r = np.ascontiguousarray(np.broadcast_to(bv, (AG, DV)), dtype=np.float32)

    in_maps = []
    for c in range(NC):
        g, s = c // GS, c % GS
        gslice = slice(g * GAG, (g + 1) * GAG)
        mslice = slice(s * MS, (s + 1) * MS)
        rm_t = received_messages[gslice, mslice].T[:, _AGENT_PERM]  # [4096, 512]
        wq_sh = Wq[mslice, :] * SCALE  # [4096, 512], pre-scaled
        taus = imagined_trajectory[:, c * AG : (c + 1) * AG, :].reshape(T * AG, TAU)
        traj_t = taus.T  # [2048, 1024]
        rm_p = np.ascontiguousarray(
            rm_t.reshape(KQ, 128, GAG // 128, 128).transpose(2, 1, 0, 3),
            dtype=_cdt(),
        )
        traj_p = np.ascontiguousarray(
            traj_t.reshape(KT, 128, RT, 128).transpose(2, 1, 0, 3), dtype=_cdt()
        )
        in_maps.append(
            {
                "rm": rm_p,
                "wq": _pack(wq_sh, KQ, HID),
                "traj": traj_p,
                "wk": wk_p,
                "wv": wv_p,
                "bqs": bqs,
                "bv": bv_r,
            }
        )
    return in_maps


def kernel(
    imagined_trajectory,
    received_messages,
    Wq,
    bq,
    Wk,
    bk,
    Wv,
    bv,
):
    global LAST_EXEC_NS, LAST_RESULTS
    from concourse.bass_utils import run_bass_kernel_spmd

    if "nc" not in _CACHE:
        _CACHE["nc"] = _build()
    nc = _CACHE["nc"]

    in_maps = _make_in_maps(
        imagined_trajectory, received_messages, Wq, bq, Wk, bk, Wv, bv
    )

    res = run_bass_kernel_spmd(
        nc,
        in_maps,
        core_ids=list(range(NC)),
        trace=TRACE,
        trace_cores=TRACE_CORES,
        stitch_traces=STITCH,
    )
    LAST_EXEC_NS = res.exec_time_ns
    LAST_RESULTS = res
    out = np.concatenate([res.results[c]["out"] for c in range(NC)], axis=0)
    return out.astype(np.float32)


# revision 28
# speedup vs baseline: 1.1147x; 1.0309x over previous
"""Distributed Trainium2 kernel for the per-agent trajectory attention module.

Math (per reference):
    q = received_messages @ Wq + bq                    [512, 512]
    k = taus @ Wk + bk ; v = taus @ Wv + bv            [16*512, 512/64]
    scores[i, t] = dot(q[i], k[t, i]) / sqrt(512)
    messages[i] = sum_t softmax(scores)[i, t] * v[t, i]  [512, 64]

Sharding over 8 NeuronCores:
  - q: tensor-parallel over the 32768 msg dim (4096 per core), partial q for
    ALL 512 agents per core, one f32 ReduceScatter(add) over the agent axis ->
    each core holds q for its own 64 agents.  The 1/sqrt(H) scale is folded
    into Wq on the host and bq*scale/8 into each core's pre-collective drain,
    so nothing downstream of the collective needs a fixup op.  bk drops out
    of softmax exactly (per-agent constant shift); bv is added at the end.
  - k/v/attention: data-parallel over agents (64 per core).
  - matmul operands stored/computed in bf16 (halves the DMA roofline);
    PSUM accumulation and everything after the matmuls is f32.

Scheduling notes (v2):
  - every input tile is resident in SBUF; the bulk loads all go on the sync
    HWDGE ring in priority order (rm0, wq, rm1-3, traj, wk, wv) so the q
    path -- which gates the ReduceScatter -- gets the HBM bandwidth first,
    and the latency-critical small transfers (rs_in drains, q2, merge, out)
    ride the otherwise-idle scalar HWDGE ring.
  - softmax skips the max-subtraction (scores are bounded |s| < ~20 for this
    problem so exp stays comfortably inside f32), which removes the
    cross-partition score gather; the odd/even trajectory-step halves are
    instead merged once at the very end: the unnormalized exp-weighted v sum
    and the exp row-sum live in one [128, 1+DV] tile, one tiny SBUF->SBUF
    DMA folds the upper partition half down, and a single
    (sum*recip)+bv fixup produces the output.
"""

import math

import numpy as np

T = 16
N_AGENTS = 512
TAU = 2048
MSG = 32768
HID = 512
DV = 64

NC = 8
AG = N_AGENTS // NC  # 64 agents per core
GS = 8  # group size: the q reduce-scatter spans all 8 cores
NG = NC // GS
GAG = N_AGENTS // NG  # agents per group (512)
MS = MSG // GS  # 4096 msg columns per core
KQ = MS // 128  # 32 contraction chunks for q
KT = TAU // 128  # 16 contraction chunks for k/v
RT = (T * AG) // 128  # 8 row-tiles of taus per core (128 rows each)

SCALE = 1.0 / math.sqrt(HID)

# compute/storage dtype for the big matmul operands: "f32r" (full f32 storage,
# tf32-ish matmul precision) or "bf16" (half the DMA bytes, bf16 matmuls)
DTYPE = "bf16"
WARMUP_MMS = 32  # dummy matmuls to lift the PE HAM throttle before real work
RS_DT = "f16"  # collective wire dtype: "f32" or "f16" (CCE bf16 is broken on HW)

_CACHE = {}

# set by test harness: run with trace and stash exec time here
TRACE = False
TRACE_CORES = None
STITCH = False
LAST_EXEC_NS = None
LAST_RESULTS = None


def _build():
    import concourse.bacc as bacc
    import concourse.mybir as mybir
    import concourse.tile as tile

    f32 = mybir.dt.float32
    f32r = mybir.dt.float32r if DTYPE == "f32r" else mybir.dt.bfloat16
    add = mybir.AluOpType.add
    mult = mybir.AluOpType.mult

    nc = bacc.Bacc("TRN2", target_bir_lowering=False, debug=False, num_devices=NC)

    # inputs (per-core shards, pre-packed host-side; layout [128, kc, n])
    rm_d = nc.dram_tensor("rm", [GAG // 128, 128, KQ, 128], f32r, kind="ExternalInput")
    wq_d = nc.dram_tensor("wq", [128, KQ, HID], f32r, kind="ExternalInput")
    traj_d = nc.dram_tensor("traj", [RT, 128, KT, 128], f32r, kind="ExternalInput")
    wk_d = nc.dram_tensor("wk", [128, KT, HID], f32r, kind="ExternalInput")
    wv_d = nc.dram_tensor("wv", [128, KT, DV], f32r, kind="ExternalInput")
    bqs_d = nc.dram_tensor("bqs", [128, HID], f32, kind="ExternalInput")  # bq * SCALE
    bv_d = nc.dram_tensor("bv", [AG, DV], f32, kind="ExternalInput")
    out_d = nc.dram_tensor("out", [AG, DV], f32, kind="ExternalOutput")

    # collective wire dtype (CCE bf16 reduce is broken on HW; fp16 has the
    # mantissa for this data -- q values are O(10) -- and halves the payload)
    cdt = mybir.dt.float16 if RS_DT == "f16" else f32
    # the reduce-scatter is split in two so the first half can fly while the
    # second half of the q matmuls still runs; the host-side agent
    # permutation makes each half-RS scatter rows so core c receives its own
    # agents (A: local 0..31, B: local 32..63)
    rs_inA = nc.dram_tensor("rs_inA", [GAG // 2, HID], cdt)
    rs_outA = nc.dram_tensor("rs_outA", [AG // 2, HID], cdt)
    rs_inB = nc.dram_tensor("rs_inB", [GAG // 2, HID], cdt)
    rs_outB = nc.dram_tensor("rs_outB", [AG // 2, HID], cdt)


    with tile.TileContext(nc) as tc:
        with (
            tc.tile_pool(name="res", bufs=1) as res,
            tc.tile_pool(name="work", bufs=2) as work,
            tc.tile_pool(name="qps", bufs=6, space="PSUM") as qps,
            tc.tile_pool(name="vps", bufs=2, space="PSUM") as vps,
        ):
            # ---------------- PE warm-up (HAM unthrottle) ----------------
            if WARMUP_MMS:
                wz = res.tile([128, 128], f32r)
                nc.gpsimd.memset(wz[:], 0.0)
                wacc = qps.tile([128, 512], f32, tag="acc", name="warm_acc")
                for i in range(WARMUP_MMS):
                    nc.tensor.matmul(
                        wacc[:, 0:128],
                        wz[:],
                        wz[:],
                        start=(i == 0),
                        stop=(i == WARMUP_MMS - 1),
                    )

            # ---------------- resident tensors ----------------
            # small latency-insensitive loads ride the scalar ring
            bqs_sb = res.tile([128, HID], f32)
            bv_sb = res.tile([AG, DV], f32)
            nc.scalar.dma_start(bqs_sb[:], bqs_d[:])
            nc.scalar.dma_start(bv_sb[:], bv_d[:])

            # bulk loads on the sync ring, q-path first
            wq_sb = res.tile([128, KQ, HID], f32r)
            rm_tiles = [
                res.tile([128, KQ, 128], f32r, name=f"rm_sb{m}")
                for m in range(GAG // 128)
            ]
            tj_tiles = [
                res.tile([128, KT, 128], f32r, name=f"tj{rt}") for rt in range(RT)
            ]
            wk_sb = res.tile([128, KT, HID], f32r, name="wk_sb")
            wv_sb = res.tile([128, KT, DV], f32r, name="wv_sb")

            nc.sync.dma_start(rm_tiles[0][:], rm_d[0])
            for w4 in range(8):
                nc.sync.dma_start(
                    wq_sb[:, w4 * (KQ // 8) : (w4 + 1) * (KQ // 8), :],
                    wq_d[:, w4 * (KQ // 8) : (w4 + 1) * (KQ // 8), :],
                )
            for m in range(1, GAG // 128):
                nc.sync.dma_start(rm_tiles[m][:], rm_d[m])
            # wk/wv before the traj tiles: the first k matmul needs them
            nc.sync.dma_start(wk_sb[:], wk_d[:])
            nc.sync.dma_start(wv_sb[:], wv_d[:])
            for rt in range(RT):
                nc.sync.dma_start(tj_tiles[rt][:], traj_d[rt])

            # ------- q phase: partial q for the group's agents -------
            def rs_launch(in_t, out_t):
                nc.gpsimd.collective_compute(
                    "ReduceScatter",
                    add,
                    replica_groups=[list(range(NC))],
                    ins=[in_t.ap().opt()],
                    outs=[out_t.ap().opt()],
                )

            for m in range(GAG // 128):
                rm_sb = rm_tiles[m]
                qacc = qps.tile([128, HID], f32, tag="acc")
                for kc in range(KQ):
                    nc.tensor.matmul(
                        qacc[:],
                        rm_sb[:, kc, :],
                        wq_sb[:, kc, :],
                        start=(kc == 0),
                        stop=(kc == KQ - 1),
                    )
                qdr = work.tile([128, HID], cdt, tag="qdr")
                # qacc is already scaled (Wq pre-scaled on host); add bq*SCALE/NC
                # here so the ReduceScatter sum carries the bias exactly once
                nc.vector.scalar_tensor_tensor(
                    qdr[:], qacc[:], 1.0, bqs_sb[:], mult, add
                )
                half_in = rs_inA if m < 2 else rs_inB
                nc.scalar.dma_start(
                    half_in[(m % 2) * 128 : (m % 2 + 1) * 128, :], qdr[:]
                )
                if m == 1:
                    rs_launch(rs_inA, rs_outA)
                elif m == 3:
                    rs_launch(rs_inB, rs_outB)

            # local q, duplicated into both partition halves, kept in the fp16
            # wire format.  A-half loads fly while the B collective runs.
            q2 = res.tile([128, HID], cdt)
            nc.scalar.dma_start(q2[0 : AG // 2, :], rs_outA[:])
            nc.scalar.dma_start(q2[AG : AG + AG // 2, :], rs_outA[:])
            nc.scalar.dma_start(q2[AG // 2 : AG, :], rs_outB[:])
            nc.scalar.dma_start(q2[AG + AG // 2 : 128, :], rs_outB[:])

            # ---------------- k/v phase + scores ----------------
            # v_sb column 0 is a constant 1.0: the exp-weighted v chain then
            # accumulates sum(exp) in comb[:,0] for free
            v_sb = res.tile([128, RT, 1 + DV], f32)
            nc.gpsimd.memset(v_sb[:, :, 0:1], 1.0)
            k_sb = res.tile([128, RT, HID], cdt)
            s_scr = res.tile([128, RT], f32)

            def kv_tile(rt):
                tj_sb = tj_tiles[rt]
                kacc = qps.tile([128, HID], f32, tag="acc", name=f"kacc{rt}")
                for kc in range(KT):
                    nc.tensor.matmul(
                        kacc[:],
                        tj_sb[:, kc, :],
                        wk_sb[:, kc, :],
                        start=(kc == 0),
                        stop=(kc == KT - 1),
                    )
                vacc = vps.tile([128, DV], f32, tag="vacc", name=f"vacc{rt}")
                for kc in range(KT):
                    nc.tensor.matmul(
                        vacc[:],
                        tj_sb[:, kc, :],
                        wv_sb[:, kc, :],
                        start=(kc == 0),
                        stop=(kc == KT - 1),
                    )
                # park k in SBUF so the PSUM bank frees without waiting on q2
                nc.vector.tensor_copy(k_sb[:, rt, :], kacc[:])
                return nc.vector.tensor_copy(v_sb[:, rt, 1:], vacc[:])

            def score_tile(rt):
                prod = work.tile([128, HID], cdt, tag="ttr", name=f"prod{rt}")
                return nc.vector.scalar_tensor_tensor(
                    prod[:],
                    k_sb[:, rt, :],
                    1.0,
                    q2[:],
                    mult,
                    mult,
                    accum_out=s_scr[:, rt : rt + 1],
                )

            # all k/v drains before any score: the scores stall on the
            # collective, and anything emitted after them on VectorE would
            # stall too (engine queues are FIFO)
            for rt in range(RT):
                kv_tile(rt)
            for rt in range(RT):
                score_tile(rt)

            # ------- softmax over t (16 steps per agent), no max-shift -------
            # scores for this problem are bounded (|s| < ~20), so exp is safe
            # in f32 without the running-max subtraction; the normalization
            # happens once at the very end.  The ones-column in v_sb makes
            # the same weighted chain accumulate sum(exp) in comb[:,0].
            ex = res.tile([128, RT], f32)
            comb = res.tile([128, 1 + DV], f32)  # col 0: sum(exp); 1..: sum(exp*v)
            nc.scalar.activation(
                ex[:], s_scr[:], mybir.ActivationFunctionType.Exp
            )
            nc.vector.tensor_scalar_mul(comb[:], v_sb[:, 0, :], ex[:, 0:1])
            for rt in range(1, RT):
                nc.vector.scalar_tensor_tensor(
                    comb[:], v_sb[:, rt, :], ex[:, rt : rt + 1], comb[:],
                    mult, add,
                )

            # fold the odd-step partition half onto the even half, normalize,
            # add bv, and write out — one tiny DMA, three DVE ops, one store
            combU = res.tile([AG, 1 + DV], f32)
            nc.gpsimd.dma_start(combU[:], comb[AG:128, :])
            tot = res.tile([AG, 1 + DV], f32)
            nc.vector.tensor_tensor(tot[:], comb[0:AG, :], combU[:], add)
            rcp = res.tile([AG, 1], f32)
            nc.vector.reciprocal(rcp[:], tot[:, 0:1])
            mfin = res.tile([AG, DV], f32)
            nc.vector.scalar_tensor_tensor(
                mfin[:], tot[:, 1:], rcp[:, 0:1], bv_sb[:], mult, add
            )
            nc.scalar.dma_start(out_d[:], mfin[:])

    nc.compile()
    return nc


# packed q row r -> original agent, chosen so each half reduce-scatter
# ([256, HID] over 8 ranks -> 32 rows per core) lands core c's own agents:
# half A rows c*32+j -> agent 64c+j, half B rows c*32+j -> agent 64c+32+j
_AGENT_PERM = np.array(
    [
        64 * ((r % 256) // 32) + 32 * (r // 256) + (r % 32)
        for r in range(N_AGENTS)
    ],
    dtype=np.int64,
)


def _cdt():
    if DTYPE == "bf16":
        import ml_dtypes

        return ml_dtypes.bfloat16
    return np.float32


def _pack(a, kchunks, inner):
    # [K, N] -> [128, K//128, N] with the contraction dim on partitions
    return np.ascontiguousarray(
        a.reshape(kchunks, 128, inner).transpose(1, 0, 2), dtype=_cdt()
    )


def _make_in_maps(
    imagined_trajectory, received_messages, Wq, bq, Wk, bk, Wv, bv
):
    imagined_trajectory = np.asarray(imagined_trajectory, dtype=np.float32)
    received_messages = np.asarray(received_messages, dtype=np.float32)
    Wq = np.asarray(Wq, dtype=np.float32)
    bq = np.asarray(bq, dtype=np.float32)
    Wk = np.asarray(Wk, dtype=np.float32)
    Wv = np.asarray(Wv, dtype=np.float32)
    bv = np.asarray(bv, dtype=np.float32)

    wk_p = _pack(Wk, KT, HID)
    wv_p = _pack(Wv, KT, DV)
    bqs = np.ascontiguousarray(
        np.broadcast_to(bq * SCALE / NC, (128, HID)), dtype=np.float32
    )
    bv_r = np.ascontiguousarray(np.broadcast_to(bv, (AG, DV)), dtype=np.float32)

    in_maps = []
    for c in range(NC):
        g, s = c // GS, c % GS
        gslice = slice(g * GAG, (g + 1) * GAG)
        mslice = slice(s * MS, (s + 1) * MS)
        rm_t = received_messages[gslice, mslice].T[:, _AGENT_PERM]  # [4096, 512]
        wq_sh = Wq[mslice, :] * SCALE  # [4096, 512], pre-scaled
        taus = imagined_trajectory[:, c * AG : (c + 1) * AG, :].reshape(T * AG, TAU)
        traj_t = taus.T  # [2048, 1024]
        rm_p = np.ascontiguousarray(
            rm_t.reshape(KQ, 128, GAG // 128, 128).transpose(2, 1, 0, 3),
            dtype=_cdt(),
        )
        traj_p = np.ascontiguousarray(
            traj_t.reshape(KT, 128, RT, 128).transpose(2, 1, 0, 3), dtype=_cdt()
        )
        in_maps.append(
            {
                "rm": rm_p,
                "wq": _pack(wq_sh, KQ, HID),
                "traj": traj_p,
                "wk": wk_p,
                "wv": wv_p,
                "bqs": bqs,
                "bv": bv_r,
            }
        )
    return in_maps


def kernel(
    imagined_trajectory,
    received_messages,
    Wq,
    bq,
    Wk,
    bk,
    Wv,
    bv,
):
    global LAST_EXEC_NS, LAST_RESULTS
    from concourse.bass_utils import run_bass_kernel_spmd

    if "nc" not in _CACHE:
        _CACHE["nc"] = _build()
    nc = _CACHE["nc"]

    in_maps = _make_in_maps(
        imagined_trajectory, received_messages, Wq, bq, Wk, bk, Wv, bv
    )

    res = run_bass_kernel_spmd(
        nc,
        in_maps,
        core_ids=list(range(NC)),
        trace=TRACE,
        trace_cores=TRACE_CORES,
        stitch_traces=STITCH,
    )
    LAST_EXEC_NS = res.exec_time_ns
    LAST_RESULTS = res
    out = np.concatenate([res.results[c]["out"] for c in range(NC)], axis=0)
    return out.astype(np.float32)
